# revision 2
# baseline (speedup 1.0000x reference)
"""CutOut kernel for Trainium2 (Bass), data-parallel over 8 NeuronCores.

Problem: images [64, 512, 512, 3] f32; per-sample integer centers (cy, cx);
length L (50). Output = images with the (clipped) LxL square at each
sample's center set to 0.0.

Strategy (single value-independent NEFF, pure DMA, flat layout):
  - Shard batch 64 -> 8 samples per core (pure data parallel).
  - Phase 1: DRAM->DRAM contiguous chunk copies img -> out. Each byte
    crosses an SDMA engine exactly once (no SBUF round trip, no masks,
    no compute). Chunks alternate across both HWDGE rings (sync+scalar).
  - Phase 2: per sample, one small 2D DMA overwrites a static-shape
    [2h, 2h*C] block whose top-left offset is loaded at runtime from a
    host-computed offsets tensor (value-dependent *data*, not code).
    The block source is also host-computed: zeros on cells inside the
    true (clipped) cutout window, original image values elsewhere
    (border clamping), so no clipping logic is needed on device.
    Each fix-up waits on the copy chunks covering its sample.
"""

import numpy as np

B, H, W, C = 64, 512, 512, 3
N_CORES = 8
BPC = B // N_CORES  # samples per core
WC = W * C  # 1536 floats per image row
SAMP = H * WC  # floats per sample
TOT = BPC * SAMP  # floats per core

_cache = {}


def _build_bass(zr, zc, nchunk):
    """Program for fix-up block shape [zr, zc] (floats) and nchunk copies."""
    from contextlib import ExitStack

    import concourse.bass as bass
    import concourse.mybir as mybir

    nc = bass.Bass("TRN2", target_bir_lowering=False, debug=False)
    img = nc.dram_tensor("img", [TOT], mybir.dt.float32, kind="ExternalInput")
    # one row of slack so the runtime bounds check on the dynamic fix-up
    # DMA can never trip on the extreme corner offset
    out = nc.dram_tensor("out", [TOT + WC], mybir.dt.float32, kind="ExternalOutput")
    have_fix = zr > 0 and zc > 0
    if have_fix:
        blocks = nc.dram_tensor(
            "blocks", [BPC, zr * zc], mybir.dt.float32, kind="ExternalInput"
        )
        offs = nc.dram_tensor("offs", [1, BPC], mybir.dt.int32, kind="ExternalInput")

    img_ap = img.ap()
    out_ap = out.ap()

    CH = TOT // nchunk  # floats per copy chunk (chunks per sample: nchunk/BPC)
    assert TOT % nchunk == 0 and nchunk % BPC == 0
    chunks_per_samp = nchunk // BPC
    MAXOFF = TOT - (zr - 1) * WC - zc  # block stays inside the canvas

    with ExitStack() as ctx:
        # One semaphore per sample, waited at its FULL count: a partial
        # cumulative threshold does not guarantee *which* chunk DMAs
        # completed under per-engine skew, but sem == max implies every
        # descriptor of every chunk of that sample landed.
        sampsems = [
            ctx.enter_context(nc.semaphore(f"sampsem{s}")) for s in range(BPC)
        ]
        fixsem = ctx.enter_context(nc.semaphore("fixsem"))
        if have_fix:
            offsem = ctx.enter_context(nc.semaphore("offsem"))
            off_sb = ctx.enter_context(
                nc.sbuf_tensor("off_sb", [1, BPC], mybir.dt.int32)
            )
            # gpsimd (SWDGE) ring: keeps both HWDGE rings free to start
            # streaming copy chunks immediately
            nc.gpsimd.dma_start(off_sb[0:1, :], offs.ap()).then_inc(offsem, 16)

        cps = chunks_per_samp
        for i in range(nchunk):
            eng = nc.sync if i % 2 == 0 else nc.scalar
            eng.dma_start(
                out_ap[i * CH : (i + 1) * CH], img_ap[i * CH : (i + 1) * CH]
            ).then_inc(sampsems[i // cps], 16)

        if have_fix:
            nc.sync.wait_ge(offsem, 16)
            blocks_ap = blocks.ap()
            for s in range(BPC):
                tmp = nc.sync.alloc_register(f"offreg{s}")
                nc.sync.reg_load(tmp, off_sb[0:1, s : s + 1])
                val = nc.sync.snap(tmp, donate=True)
                val = nc.s_assert_within(val, 0, MAXOFF, skip_runtime_assert=True)
                dst = bass.AP(
                    tensor=out_ap.tensor, offset=val, ap=[(WC, zr), (1, zc)]
                )
                z = nc.sync.dma_start(dst, blocks_ap[s, :])
                z.wait_op(sampsems[s], 16 * cps, "sem-ge")
                z.then_inc(fixsem, 16)
                nc.free_register(val.val)
            nc.sync.wait_ge(fixsem, 16 * BPC)
        else:
            for s in range(BPC):
                nc.sync.wait_ge(sampsems[s], 16 * cps)

    return nc


def _host_blocks(imgs, cy, cx, half, zr, zc):
    """Per-sample fix-up blocks + flat element offsets (clamped top-left).

    imgs: [B, H, WC] f32. Returns (blocks [B, zr*zc] f32, offs [B] int32),
    where block = image content at the clamped window with zeros on cells
    inside the true clipped cutout window.
    """
    zrows, zcols = zr, zc // C
    top = np.clip(cy - half, 0, H - zrows)  # [B]
    left = np.clip(cx - half, 0, W - zcols)  # [B]
    blocks = np.empty((B, zr * zc), dtype=np.float32)
    for b in range(B):
        t, l = int(top[b]), int(left[b])
        blk = imgs[b, t : t + zrows, l * C : l * C + zc].copy()  # [zr, zc]
        y0, y1 = max(int(cy[b]) - half, 0), min(int(cy[b]) + half, H)
        x0, x1 = max(int(cx[b]) - half, 0), min(int(cx[b]) + half, W)
        if y0 < y1 and x0 < x1:
            blk[y0 - t : y1 - t, (x0 - l) * C : (x1 - l) * C] = 0.0
        blocks[b] = blk.reshape(-1)
    offs = (top.astype(np.int64) * WC + left.astype(np.int64) * C).astype(np.int32)
    return blocks, offs


def kernel(images, center_y, center_x, length):
    from concourse.bass_utils import run_bass_kernel_spmd

    images = np.asarray(images)
    out_dtype = images.dtype
    cy = np.asarray(center_y).astype(np.int64)
    cx = np.asarray(center_x).astype(np.int64)
    half = int(length) // 2

    imgs = np.ascontiguousarray(images.reshape(B, H, WC), dtype=np.float32)

    zrows = min(2 * half, H)
    zcols = min(2 * half, W)
    zr, zc = zrows, zcols * C
    NCHUNK = 32

    key = (zr, zc, NCHUNK)
    if key not in _cache:
        _cache[key] = _build_bass(zr, zc, NCHUNK)
    nc = _cache[key]

    in_maps = []
    if zr > 0 and zc > 0:
        blocks, offs = _host_blocks(imgs, cy, cx, half, zr, zc)
        for c in range(N_CORES):
            sl = slice(c * BPC, (c + 1) * BPC)
            # offsets are per-core (within the core's flat [TOT] canvas)
            off_core = (
                offs[sl].astype(np.int64) + np.arange(BPC, dtype=np.int64) * SAMP
            ).astype(np.int32)
            in_maps.append(
                {
                    "img": imgs[sl].reshape(-1),
                    "blocks": blocks[sl],
                    "offs": off_core.reshape(1, BPC),
                }
            )
    else:
        for c in range(N_CORES):
            sl = slice(c * BPC, (c + 1) * BPC)
            in_maps.append({"img": imgs[sl].reshape(-1)})

    res = run_bass_kernel_spmd(nc, in_maps, core_ids=list(range(N_CORES)))
    full = np.concatenate(
        [r["out"][:TOT].reshape(BPC, H, W, C) for r in res.results], axis=0
    )
    return full.astype(out_dtype, copy=False)
